# revision 1
# baseline (speedup 1.0000x reference)
"""Multi-head attention (B=4, S=2048, D=1024, H=16, causal) on 8 TRN2 NeuronCores.

Sharding: batch (4) x head-group (2 groups of 8 heads) = 8 cores.
Megatron-style: wq/wk/wv column-parallel, wo row-parallel; the 2-way partial-sum
of the row-parallel output projection is folded into the host-side unshard.

Per-core algorithm (heads h in the core's group, q-chunks of 512 queries):
  QT[dk, s], KT[dk, s] = (x @ w + b)^T via PE matmuls on host-pre-transposed
  inputs; V[s, dv] likewise, with a ones-column appended per head so that the
  PV matmul also produces softmax denominators.
  scoresT[k, q] = KT-slices x QT (two heads packed in the 128-partition dim,
  concurrent via PE row tiling since dk=64).
  E = exp(scoresT/8) on ACT (no max-subtraction needed: scores ~ N(0,1)).
  Causality: fully-masked key-blocks are never computed; diagonal-crossing
  blocks are trapezoid-sliced to their live q-range and only the leading
  128-column triangle gets a mask multiply.
  ctxT[dv, q] accumulates V-slices x E in PSUM; row 64 = sum(E).
  Normalize: approx-reciprocal (DVE) -> partition_broadcast (GPSIMD) ->
  multiply (DVE).
  y_partial[s, do] = sum over head-pairs of ctxT-slices x wo-rows (PSUM accum).

All matmul operands are float32r (TF32-like, ~1.5 cyc/row, ~1.4e-4 rounding).
"""
import sys
import numpy as np

sys.path.insert(0, "/opt/trn_rl_repo")

from contextlib import ExitStack

import concourse.bacc as bacc
import concourse.tile as tile
from concourse import mybir
from concourse.bass_utils import run_bass_kernel_spmd

F32 = mybir.dt.float32
F32R = mybir.dt.float32r

B, S, D, H = 4, 2048, 1024, 16
DK = D // H          # 64
HG = H // 2          # 8 heads per core
DG = HG * DK         # 512 columns per core group
SC = 512             # query-chunk width
KB = 128             # key-block height
N_SC = S // SC       # 4
N_KB = S // KB       # 16
N_DM = D // 128      # 8 contraction tiles for projections
N_PAIR = HG // 2     # 4 head pairs per core
EXPSCALE = 1.0 / 8.0  # 1/sqrt(DK)


def build_program():
    """Emit the SPMD Bass program (identical on all 8 cores)."""
    nc = bacc.Bacc("TRN2", target_bir_lowering=False, debug=False)

    qT_in = nc.dram_tensor("qT", [D, S], F32R, kind="ExternalInput").ap()
    kT_in = nc.dram_tensor("kT", [D, S], F32R, kind="ExternalInput").ap()
    vT_in = nc.dram_tensor("vT", [D, S], F32R, kind="ExternalInput").ap()
    wq_in = nc.dram_tensor("wq", [D, DG], F32R, kind="ExternalInput").ap()
    wk_in = nc.dram_tensor("wk", [D, DG], F32R, kind="ExternalInput").ap()
    wv_in = nc.dram_tensor("wv", [D, DG], F32R, kind="ExternalInput").ap()
    wo_in = nc.dram_tensor("wo", [DG, D], F32R, kind="ExternalInput").ap()
    bq_in = nc.dram_tensor("bq", [1, DG], F32R, kind="ExternalInput").ap()
    bk_in = nc.dram_tensor("bk", [1, DG], F32R, kind="ExternalInput").ap()
    bv_in = nc.dram_tensor("bv", [1, DG], F32R, kind="ExternalInput").ap()
    ones_in = nc.dram_tensor("ones", [1, SC], F32R, kind="ExternalInput").ap()
    vones_in = nc.dram_tensor("vones", [128, HG], F32R, kind="ExternalInput").ap()
    # leading-triangle causal mask: mask[ki, qj] = (qj >= ki), [128, 128]
    mask_in = nc.dram_tensor("masks", [KB, KB], F32R, kind="ExternalInput").ap()
    y_out = nc.dram_tensor("y", [S, D], F32, kind="ExternalOutput").ap()

    with tile.TileContext(nc) as tc, ExitStack() as ctx:
        stage = ctx.enter_context(tc.tile_pool(name="stage", bufs=10))
        wpool = ctx.enter_context(tc.tile_pool(name="wpool", bufs=24))
        wopool = ctx.enter_context(tc.tile_pool(name="wopool", bufs=1))
        qtpool = ctx.enter_context(tc.tile_pool(name="qtpool", bufs=8))
        ktpool = ctx.enter_context(tc.tile_pool(name="ktpool", bufs=1))
        vpool = ctx.enter_context(tc.tile_pool(name="vpool", bufs=1))
        epool = ctx.enter_context(tc.tile_pool(name="epool", bufs=2))
        cpool = ctx.enter_context(tc.tile_pool(name="cpool", bufs=4))
        mpool = ctx.enter_context(tc.tile_pool(name="mpool", bufs=1))
        ypool = ctx.enter_context(tc.tile_pool(name="ypool", bufs=2))
        rpool = ctx.enter_context(tc.tile_pool(name="rpool", bufs=2))
        bpool = ctx.enter_context(tc.tile_pool(name="bpool", bufs=2))
        onepool = ctx.enter_context(tc.tile_pool(name="onepool", bufs=1))
        pspool = ctx.enter_context(tc.tile_pool(name="pspool", bufs=1, space="PSUM"))

        # ---- constants ----
        ones_sb = onepool.tile([1, SC], F32R, name="ones_sb")
        nc.sync.dma_start(ones_sb[:], ones_in[:])
        bq_sb = onepool.tile([1, DG], F32R, name="bq_sb")
        nc.sync.dma_start(bq_sb[:], bq_in[:])
        bk_sb = onepool.tile([1, DG], F32R, name="bk_sb")
        nc.sync.dma_start(bk_sb[:], bk_in[:])
        bv_sb = onepool.tile([1, DG], F32R, name="bv_sb")
        nc.sync.dma_start(bv_sb[:], bv_in[:])
        mask_sb = mpool.tile([KB, KB], F32R, name="mask_sb")
        nc.sync.dma_start(mask_sb[:], mask_in[:])

        # ---- persistent data regions ----
        # KT: per (head-pair, s-chunk) tile [128, SC]; rows 0:64 head 2p.
        KT = [[ktpool.tile([128, SC], F32R, name=f"KT{p}_{sc}") for sc in range(N_SC)]
              for p in range(N_PAIR)]
        # V: per key-block tile [128, HG, 65]; per head 64 value cols + ones col.
        V = [vpool.tile([128, HG, 65], F32R, name=f"V{kb}") for kb in range(N_KB)]
        for kb in range(N_KB):
            nc.sync.dma_start(V[kb][:, :, 64:65], vones_in.unsqueeze(-1))

        w_sb = {}
        for nm, w_in in (("wq", wq_in), ("wk", wk_in), ("wv", wv_in)):
            w_sb[nm] = []
            for dm in range(N_DM):
                t = wpool.tile([128, DG], F32R, name=f"{nm}_{dm}", tag="w")
                nc.sync.dma_start(t[:], w_in[dm * 128:(dm + 1) * 128, :])
                w_sb[nm].append(t)
        wo_sb = []
        for p in range(N_PAIR):
            t = wopool.tile([128, D], F32R, name=f"wo_{p}")
            nc.sync.dma_start(t[:], wo_in[p * 128:(p + 1) * 128, :])
            wo_sb.append(t)

        def ps_small(name):
            return pspool.tile([128, SC], F32, name=name, tag="psa", bufs=2)

        def stage_chunk(nm, xT_in, sc):
            xs = []
            for dm in range(N_DM):
                t = stage.tile([128, SC], F32R, name=f"{nm}s{sc}_{dm}", tag="stage")
                nc.sync.dma_start(
                    t[:], xT_in[dm * 128:(dm + 1) * 128, sc * SC:(sc + 1) * SC]
                )
                xs.append(t)
            return xs

        def proj_qk(nm, xs, bias, dst_tiles):
            # dst_tiles[p] <- [128, SC] transposed projection of this chunk
            for p in range(N_PAIR):
                ps = ps_small(f"ps_{nm}")
                nc.tensor.matmul(
                    ps[:], bias[0:1, p * 128:(p + 1) * 128], ones_sb[0:1, :],
                    start=True, stop=False,
                )
                for dm in range(N_DM):
                    nc.tensor.matmul(
                        ps[:],
                        w_sb["w" + nm][dm][:, p * 128:(p + 1) * 128],
                        xs[dm][:],
                        start=False, stop=(dm == N_DM - 1),
                    )
                nc.vector.tensor_copy(dst_tiles[p][:], ps[:])

        # ---- main loop: project chunk sc, then attention for q-chunk sc ----
        for sc in range(N_SC):
            xs = stage_chunk("q", qT_in, sc)
            QTc = [qtpool.tile([128, SC], F32R, name=f"QT{p}_{sc}", tag="qtc")
                   for p in range(N_PAIR)]
            proj_qk("q", xs, bq_sb, QTc)

            xs = stage_chunk("k", kT_in, sc)
            proj_qk("k", xs, bk_sb, [KT[p][sc] for p in range(N_PAIR)])

            xs = stage_chunk("v", vT_in, sc)
            for sb in range(4):
                ps = ps_small("ps_v")
                nc.tensor.matmul(
                    ps[:], ones_sb[0:1, 0:128], bv_sb[0:1, :],
                    start=True, stop=False,
                )
                for dm in range(N_DM):
                    nc.tensor.matmul(
                        ps[:],
                        xs[dm][:, sb * 128:(sb + 1) * 128],
                        w_sb["wv"][dm][:],
                        start=False, stop=(dm == N_DM - 1),
                    )
                kb = sc * 4 + sb
                nc.vector.tensor_copy(
                    V[kb][:, :, 0:64], ps[:].rearrange("p (h d) -> p h d", h=HG)
                )

            # ---- attention for q-chunk qc = sc ----
            qc = sc
            kbmax = 4 * (qc + 1)
            ctx_pairs = []
            for p in range(N_PAIR):
                h0, h1 = 2 * p, 2 * p + 1
                ctx0 = pspool.tile([65, SC], F32, name="ctx0", tag="psctx0", bufs=1)
                ctx1 = pspool.tile([65, SC], F32, name="ctx1", tag="psctx1", bufs=1)
                for kb in range(kbmax):
                    j = kb - 4 * qc  # >=0: diagonal-crossing block
                    off = max(j, 0) * KB  # live q-range is [off, SC)
                    kt = KT[p][kb // 4]
                    kcol = (kb % 4) * KB
                    scps = pspool.tile([128, 2 * SC], F32, name="scps", tag="pssc",
                                       bufs=2)
                    # scoresT: two heads concurrently via PE row groups
                    nc.tensor.matmul(
                        scps[:, off:SC],
                        kt[0:64, kcol:kcol + KB],
                        QTc[p][0:64, off:SC],
                        start=True, stop=True,
                    )
                    nc.tensor.matmul(
                        scps[:, SC + off:2 * SC],
                        kt[64:128, kcol:kcol + KB],
                        QTc[p][64:128, off:SC],
                        start=True, stop=True,
                    )
                    e = epool.tile([128, 2 * SC], F32R, name="e", tag="e")
                    if off == 0:
                        nc.scalar.activation(
                            e[:], scps[:], mybir.ActivationFunctionType.Exp,
                            scale=EXPSCALE,
                        )
                    else:
                        nc.scalar.activation(
                            e[:, off:SC], scps[:, off:SC],
                            mybir.ActivationFunctionType.Exp, scale=EXPSCALE,
                        )
                        nc.scalar.activation(
                            e[:, SC + off:2 * SC], scps[:, SC + off:2 * SC],
                            mybir.ActivationFunctionType.Exp, scale=EXPSCALE,
                        )
                    if j >= 0:  # mask the leading 128-col triangle
                        nc.vector.tensor_mul(e[:, off:off + KB], e[:, off:off + KB],
                                             mask_sb[:])
                        nc.vector.tensor_mul(e[:, SC + off:SC + off + KB],
                                             e[:, SC + off:SC + off + KB], mask_sb[:])
                    first, last = kb == 0, kb == kbmax - 1
                    nc.tensor.matmul(
                        ctx0[:, off:SC], V[kb][:, h0, :], e[:, off:SC],
                        start=first, stop=last,
                    )
                    nc.tensor.matmul(
                        ctx1[:, off:SC], V[kb][:, h1, :], e[:, SC + off:2 * SC],
                        start=first, stop=last,
                    )
                # normalize: ctx rows 0:64 / ctx row 64
                cp = cpool.tile([128, SC], F32R, name="cp", tag="ctx")
                for i, cps in ((0, ctx0), (1, ctx1)):
                    # reciprocal_approx_fast mis-reads PSUM on HW: hop via SBUF
                    srow = rpool.tile([1, SC], F32, name="srow", tag="rec", bufs=2)
                    nc.vector.tensor_copy(srow[:], cps[64:65, :])
                    rec = rpool.tile([1, SC], F32, name="rec", tag="rec", bufs=2)
                    nc.vector.reciprocal_approx_fast(rec[:], srow[:])
                    rb = bpool.tile([64, SC], F32, name="rb", tag="rb", bufs=2)
                    nc.gpsimd.partition_broadcast(rb[:], rec[:])
                    nc.vector.tensor_tensor(
                        cp[i * 64:(i + 1) * 64, :], cps[0:64, :], rb[:],
                        mybir.AluOpType.mult,
                    )
                ctx_pairs.append(cp)

            # output projection for this q-chunk
            for sb in range(4):
                yst = ypool.tile([128, D], F32, name="yst", tag="y")
                for dc in range(2):
                    yps = ps_small("yps")
                    for p in range(N_PAIR):
                        nc.tensor.matmul(
                            yps[:],
                            ctx_pairs[p][:, sb * 128:(sb + 1) * 128],
                            wo_sb[p][:, dc * SC:(dc + 1) * SC],
                            start=(p == 0), stop=(p == N_PAIR - 1),
                        )
                    nc.vector.tensor_copy(yst[:, dc * SC:(dc + 1) * SC], yps[:])
                row = qc * SC + sb * 128
                nc.sync.dma_start(y_out[row:row + 128, :], yst[:])

    nc.compile()
    return nc


def make_inputs(q, k, v, wq, bq, wk, bk, wv, bv, wo):
    """Host-side shard + layout prep. Returns list of 8 per-core input dicts."""
    f = np.float32
    qj = np.arange(KB)[None, :]
    ki = np.arange(KB)[:, None]
    mask = np.ascontiguousarray((qj >= ki).astype(f))
    ones = np.ones((1, SC), f)
    vones = np.ones((128, HG), f)

    qT = [np.ascontiguousarray(np.asarray(q[b]).T) for b in range(B)]
    kT = [np.ascontiguousarray(np.asarray(k[b]).T) for b in range(B)]
    vT = [np.ascontiguousarray(np.asarray(v[b]).T) for b in range(B)]

    in_maps = []
    for c in range(8):
        b, g = c // 2, c % 2
        sl = slice(g * DG, (g + 1) * DG)
        in_maps.append({
            "qT": qT[b], "kT": kT[b], "vT": vT[b],
            "wq": np.ascontiguousarray(wq[:, sl]),
            "wk": np.ascontiguousarray(wk[:, sl]),
            "wv": np.ascontiguousarray(wv[:, sl]),
            "wo": np.ascontiguousarray(wo[sl, :]),
            "bq": np.ascontiguousarray(bq[sl]).reshape(1, DG),
            "bk": np.ascontiguousarray(bk[sl]).reshape(1, DG),
            "bv": np.ascontiguousarray(bv[sl]).reshape(1, DG),
            "ones": ones, "vones": vones, "masks": mask,
        })
    return in_maps


def combine_outputs(results, bo):
    """Sum the two row-parallel partials per batch and add the output bias."""
    out = np.empty((B, S, D), np.float32)
    for b in range(B):
        out[b] = results[2 * b]["y"] + results[2 * b + 1]["y"] + np.asarray(bo)[None, :]
    return out


_NC_CACHE = {}


def kernel(x, q, k, v, mask, wq, bq, wk, bk, wv, bv, wo, bo):
    # x is unused (overwritten in the reference forward); mask is the causal
    # tril mask, which is hardcoded in the on-device masking.
    if "nc" not in _NC_CACHE:
        _NC_CACHE["nc"] = build_program()
    nc = _NC_CACHE["nc"]
    in_maps = make_inputs(q, k, v, wq, bq, wk, bk, wv, bv, wo)
    r = run_bass_kernel_spmd(nc, in_maps, core_ids=list(range(8)))
    return combine_outputs(r.results, bo)



# revision 2
# speedup vs baseline: 214.3983x; 214.3983x over previous
"""Multi-head attention (B=4, S=2048, D=1024, H=16, causal) on 8 TRN2 NeuronCores.

Sharding: batch (4) x head-group (2 groups of 8 heads) = 8 cores.
Megatron-style: wq/wk/wv column-parallel, wo row-parallel; the 2-way partial-sum
of the row-parallel output projection is folded into the host-side unshard.

v2: all matmul operands bf16 (host-side cast; output tolerance 2e-2 dwarfs the
~4e-3 bf16 rounding).  bf16 enables Fast Weight Load so the per-matmul
LDWEIGHTS hides behind the matmul stream, and halves DMA + SBUF traffic.
Staging DMAs are one 3D-AP transfer per (tensor, chunk) and are emitted in
first-use order so the first projection starts ~5us in instead of ~45us.
q/k bias-adds are folded into the PSUM->SBUF copy on DVE (per-partition scalar
broadcast) instead of burning 512-row PE matmuls.

Per-core algorithm (heads h in the core's group, q-chunks of 512 queries):
  QT[dk, s], KT[dk, s] = (x @ w + b)^T via PE matmuls on host-pre-transposed
  inputs; V[s, dv] likewise, with a ones-column appended per head so that the
  PV matmul also produces softmax denominators.
  scoresT[k, q] = KT-slices x QT (two heads packed in the 128-partition dim,
  concurrent via PE row tiling since dk=64).
  E = exp(scoresT/8) on ACT (no max-subtraction needed: scores ~ N(0,1)).
  Causality: fully-masked key-blocks are never computed; diagonal-crossing
  blocks are trapezoid-sliced to their live q-range and only the leading
  128-column triangle gets a mask multiply.
  ctxT[dv, q] accumulates V-slices x E in PSUM; row 64 = sum(E).
  Normalize: approx-reciprocal (DVE) -> partition_broadcast (GPSIMD) ->
  multiply (DVE).
  y_partial[s, do] = sum over head-pairs of ctxT-slices x wo-rows (PSUM accum).
"""
import sys
import numpy as np

sys.path.insert(0, "/opt/trn_rl_repo")

from contextlib import ExitStack

import ml_dtypes

import concourse.bacc as bacc
import concourse.tile as tile
from concourse import mybir
from concourse.bass_utils import run_bass_kernel_spmd

F32 = mybir.dt.float32
BF16 = mybir.dt.bfloat16
BF16NP = ml_dtypes.bfloat16

B, S, D, H = 4, 2048, 1024, 16
DK = D // H          # 64
HG = H // 2          # 8 heads per core
DG = HG * DK         # 512 columns per core group
SC = 512             # query-chunk width
KB = 128             # key-block height
N_SC = S // SC       # 4
N_KB = S // KB       # 16
N_DM = D // 128      # 8 contraction tiles for projections
N_PAIR = HG // 2     # 4 head pairs per core
EXPSCALE = 1.0 / 8.0  # 1/sqrt(DK)


def build_program():
    """Emit the SPMD Bass program (identical on all 8 cores)."""
    nc = bacc.Bacc("TRN2", target_bir_lowering=False, debug=False)

    qT_in = nc.dram_tensor("qT", [D, S], BF16, kind="ExternalInput").ap()
    kT_in = nc.dram_tensor("kT", [D, S], BF16, kind="ExternalInput").ap()
    vT_in = nc.dram_tensor("vT", [D, S], BF16, kind="ExternalInput").ap()
    wq_in = nc.dram_tensor("wq", [D, DG], BF16, kind="ExternalInput").ap()
    wk_in = nc.dram_tensor("wk", [D, DG], BF16, kind="ExternalInput").ap()
    wv_in = nc.dram_tensor("wv", [D, DG], BF16, kind="ExternalInput").ap()
    wo_in = nc.dram_tensor("wo", [DG, D], BF16, kind="ExternalInput").ap()
    # bqkT[:, 0:4] = bq pair-columns, [:, 4:8] = bk pair-columns
    bqkT_in = nc.dram_tensor("bqkT", [128, 2 * N_PAIR], F32,
                             kind="ExternalInput").ap()
    bv_in = nc.dram_tensor("bv", [1, DG], BF16, kind="ExternalInput").ap()
    # leading-triangle causal mask: mask[ki, qj] = (qj >= ki), [128, 128]
    mask_in = nc.dram_tensor("masks", [KB, KB], BF16, kind="ExternalInput").ap()
    y_out = nc.dram_tensor("y", [S, D], F32, kind="ExternalOutput").ap()

    with tile.TileContext(nc) as tc, ExitStack() as ctx:
        stage = ctx.enter_context(tc.tile_pool(name="stage", bufs=6))
        wpool = ctx.enter_context(tc.tile_pool(name="wpool", bufs=1))
        qtpool = ctx.enter_context(tc.tile_pool(name="qtpool", bufs=8))
        ktpool = ctx.enter_context(tc.tile_pool(name="ktpool", bufs=1))
        vpool = ctx.enter_context(tc.tile_pool(name="vpool", bufs=1))
        epool = ctx.enter_context(tc.tile_pool(name="epool", bufs=3))
        cpool = ctx.enter_context(tc.tile_pool(name="cpool", bufs=4))
        mpool = ctx.enter_context(tc.tile_pool(name="mpool", bufs=1))
        ypool = ctx.enter_context(tc.tile_pool(name="ypool", bufs=2))
        rpool = ctx.enter_context(tc.tile_pool(name="rpool", bufs=2))
        bpool = ctx.enter_context(tc.tile_pool(name="bpool", bufs=2))
        onepool = ctx.enter_context(tc.tile_pool(name="onepool", bufs=1))
        pspool = ctx.enter_context(tc.tile_pool(name="pspool", bufs=1, space="PSUM"))

        # ---- persistent data regions ----
        # KT: per (head-pair, s-chunk) tile [128, SC]; rows 0:64 head 2p.
        KT = [[ktpool.tile([128, SC], BF16, name=f"KT{p}_{sc}")
               for sc in range(N_SC)] for p in range(N_PAIR)]
        # V: per key-block tile [128, HG, 65]; per head 64 value cols + ones col.
        V = [vpool.tile([128, HG, 65], BF16, name=f"V{kb}") for kb in range(N_KB)]

        # weights: one 3D tile per projection, w_sb[:, dm, :] = rows dm*128..+128
        w_sb = {}
        wo_sb = wpool.tile([128, N_PAIR, D], BF16, name="wo_sb")
        ones_sb = onepool.tile([1, 128], BF16, name="ones_sb")
        bqk_sb = onepool.tile([128, 2 * N_PAIR], F32, name="bqk_sb")
        bv_sb = onepool.tile([1, DG], BF16, name="bv_sb")
        mask_sb = mpool.tile([KB, KB], BF16, name="mask_sb")

        def stage_chunk(nm, xT_in, sc):
            # [128, dm, col] <- DRAM rows (p + 128*dm), cols sc*SC..+SC
            t = stage.tile([128, N_DM, SC], BF16, name=f"{nm}s{sc}", tag="stage")
            src = xT_in.rearrange("(dm p) s -> p dm s", p=128)
            nc.sync.dma_start(t[:], src[:, :, sc * SC:(sc + 1) * SC])
            return t

        # ---- DMA / init order: first use first ----
        w_sb["q"] = wpool.tile([128, N_DM, DG], BF16, name="wq_sb")
        nc.sync.dma_start(w_sb["q"][:], wq_in.rearrange("(dm p) c -> p dm c", p=128))
        xq0 = stage_chunk("q", qT_in, 0)
        nc.sync.dma_start(bqk_sb[:], bqkT_in[:])
        w_sb["k"] = wpool.tile([128, N_DM, DG], BF16, name="wk_sb")
        nc.sync.dma_start(w_sb["k"][:], wk_in.rearrange("(dm p) c -> p dm c", p=128))
        xk0 = stage_chunk("k", kT_in, 0)
        w_sb["v"] = wpool.tile([128, N_DM, DG], BF16, name="wv_sb")
        nc.sync.dma_start(w_sb["v"][:], wv_in.rearrange("(dm p) c -> p dm c", p=128))
        nc.sync.dma_start(bv_sb[:], bv_in[:])
        xv0 = stage_chunk("v", vT_in, 0)
        nc.sync.dma_start(mask_sb[:], mask_in[:])
        nc.sync.dma_start(wo_sb[:], wo_in.rearrange("(p r) c -> r p c", r=128))
        nc.gpsimd.memset(ones_sb[:], 1.0)
        for kb in range(N_KB):
            nc.gpsimd.memset(V[kb][:, :, 64:65], 1.0)

        def ps_small(name):
            return pspool.tile([128, SC], F32, name=name, tag="psa", bufs=2)

        def proj_qk(nm, xs, bcol, dst_tiles):
            # dst_tiles[p] <- [128, SC] transposed projection of this chunk,
            # bias added during the PSUM->SBUF copy (bcol = [128,1] slice).
            for p in range(N_PAIR):
                ps = ps_small(f"ps_{nm}")
                for dm in range(N_DM):
                    nc.tensor.matmul(
                        ps[:],
                        w_sb[nm][:, dm, p * 128:(p + 1) * 128],
                        xs[:, dm, :],
                        start=(dm == 0), stop=(dm == N_DM - 1),
                    )
                nc.vector.tensor_tensor(
                    dst_tiles[p][:], ps[:],
                    bcol[:, p:p + 1].broadcast_to([128, SC]),
                    mybir.AluOpType.add,
                )

        # ---- main loop: project chunk sc, then attention for q-chunk sc ----
        for sc in range(N_SC):
            xq = xq0 if sc == 0 else stage_chunk("q", qT_in, sc)
            QTc = [qtpool.tile([128, SC], BF16, name=f"QT{p}_{sc}", tag="qtc")
                   for p in range(N_PAIR)]
            proj_qk("q", xq, bqk_sb[:, 0:N_PAIR], QTc)

            xk = xk0 if sc == 0 else stage_chunk("k", kT_in, sc)
            proj_qk("k", xk, bqk_sb[:, N_PAIR:2 * N_PAIR],
                    [KT[p][sc] for p in range(N_PAIR)])

            xv = xv0 if sc == 0 else stage_chunk("v", vT_in, sc)
            for sb in range(4):
                ps = ps_small("ps_v")
                nc.tensor.matmul(
                    ps[:], ones_sb[0:1, :], bv_sb[0:1, :],
                    start=True, stop=False,
                )
                for dm in range(N_DM):
                    nc.tensor.matmul(
                        ps[:],
                        xv[:, dm, sb * 128:(sb + 1) * 128],
                        w_sb["v"][:, dm, :],
                        start=False, stop=(dm == N_DM - 1),
                    )
                kb = sc * 4 + sb
                nc.vector.tensor_copy(
                    V[kb][:, :, 0:64], ps[:].rearrange("p (h d) -> p h d", h=HG)
                )

            # ---- attention for q-chunk qc = sc ----
            qc = sc
            kbmax = 4 * (qc + 1)
            ctx_pairs = []
            for p in range(N_PAIR):
                h0, h1 = 2 * p, 2 * p + 1
                ctx0 = pspool.tile([65, SC], F32, name="ctx0", tag="psctx0", bufs=1)
                ctx1 = pspool.tile([65, SC], F32, name="ctx1", tag="psctx1", bufs=1)
                for kb in range(kbmax):
                    j = kb - 4 * qc  # >=0: diagonal-crossing block
                    off = max(j, 0) * KB  # live q-range is [off, SC)
                    kt = KT[p][kb // 4]
                    kcol = (kb % 4) * KB
                    scps = pspool.tile([128, 2 * SC], F32, name="scps", tag="pssc",
                                       bufs=2)
                    # scoresT: two heads concurrently via PE row groups
                    nc.tensor.matmul(
                        scps[:, off:SC],
                        kt[0:64, kcol:kcol + KB],
                        QTc[p][0:64, off:SC],
                        start=True, stop=True,
                    )
                    nc.tensor.matmul(
                        scps[:, SC + off:2 * SC],
                        kt[64:128, kcol:kcol + KB],
                        QTc[p][64:128, off:SC],
                        start=True, stop=True,
                    )
                    e = epool.tile([128, 2, SC], BF16, name="e", tag="e")
                    if off == 0:
                        nc.scalar.activation(
                            e[:], scps[:].rearrange("p (h q) -> p h q", h=2),
                            mybir.ActivationFunctionType.Exp,
                            scale=EXPSCALE,
                        )
                    else:
                        nc.scalar.activation(
                            e[:, :, off:SC],
                            scps[:].rearrange("p (h q) -> p h q", h=2)[:, :, off:SC],
                            mybir.ActivationFunctionType.Exp, scale=EXPSCALE,
                        )
                    if j >= 0:  # mask the leading 128-col triangle
                        nc.vector.tensor_tensor(
                            e[:, :, off:off + KB], e[:, :, off:off + KB],
                            mask_sb[:].unsqueeze(1).broadcast_to([KB, 2, KB]),
                            mybir.AluOpType.mult,
                        )
                    first, last = kb == 0, kb == kbmax - 1
                    nc.tensor.matmul(
                        ctx0[:, off:SC], V[kb][:, h0, :], e[:, 0, off:SC],
                        start=first, stop=last,
                    )
                    nc.tensor.matmul(
                        ctx1[:, off:SC], V[kb][:, h1, :], e[:, 1, off:SC],
                        start=first, stop=last,
                    )
                # normalize: ctx rows 0:64 / ctx row 64
                cp = cpool.tile([128, SC], BF16, name="cp", tag="ctx")
                for i, cps in ((0, ctx0), (1, ctx1)):
                    # reciprocal_approx_fast mis-reads PSUM on HW: hop via SBUF
                    srow = rpool.tile([1, SC], F32, name="srow", tag="rec", bufs=2)
                    nc.vector.tensor_copy(srow[:], cps[64:65, :])
                    rec = rpool.tile([1, SC], F32, name="rec", tag="rec", bufs=2)
                    nc.vector.reciprocal_approx_fast(rec[:], srow[:])
                    rb = bpool.tile([64, SC], F32, name="rb", tag="rb", bufs=2)
                    nc.gpsimd.partition_broadcast(rb[:], rec[:])
                    nc.vector.tensor_tensor(
                        cp[i * 64:(i + 1) * 64, :], cps[0:64, :], rb[:],
                        mybir.AluOpType.mult,
                    )
                ctx_pairs.append(cp)

            # output projection for this q-chunk
            for sb in range(4):
                yst = ypool.tile([128, D], F32, name="yst", tag="y")
                for dc in range(2):
                    yps = ps_small("yps")
                    for p in range(N_PAIR):
                        nc.tensor.matmul(
                            yps[:],
                            ctx_pairs[p][:, sb * 128:(sb + 1) * 128],
                            wo_sb[:, p, dc * SC:(dc + 1) * SC],
                            start=(p == 0), stop=(p == N_PAIR - 1),
                        )
                    nc.vector.tensor_copy(yst[:, dc * SC:(dc + 1) * SC], yps[:])
                row = qc * SC + sb * 128
                nc.sync.dma_start(y_out[row:row + 128, :], yst[:])

    nc.compile()
    return nc


def make_inputs(q, k, v, wq, bq, wk, bk, wv, bv, wo):
    """Host-side shard + layout prep. Returns list of 8 per-core input dicts."""
    bf = BF16NP
    qj = np.arange(KB)[None, :]
    ki = np.arange(KB)[:, None]
    mask = np.ascontiguousarray((qj >= ki).astype(bf))

    qT = [np.ascontiguousarray(np.asarray(q[b]).T.astype(bf)) for b in range(B)]
    kT = [np.ascontiguousarray(np.asarray(k[b]).T.astype(bf)) for b in range(B)]
    vT = [np.ascontiguousarray(np.asarray(v[b]).T.astype(bf)) for b in range(B)]

    in_maps = []
    for c in range(8):
        b, g = c // 2, c % 2
        sl = slice(g * DG, (g + 1) * DG)
        bqk = np.stack([np.asarray(bq[sl]).astype(np.float32)
                        .reshape(N_PAIR, 128)[p] for p in range(N_PAIR)]
                       + [np.asarray(bk[sl]).astype(np.float32)
                          .reshape(N_PAIR, 128)[p] for p in range(N_PAIR)],
                       axis=1)
        in_maps.append({
            "qT": qT[b], "kT": kT[b], "vT": vT[b],
            "wq": np.ascontiguousarray(np.asarray(wq)[:, sl].astype(bf)),
            "wk": np.ascontiguousarray(np.asarray(wk)[:, sl].astype(bf)),
            "wv": np.ascontiguousarray(np.asarray(wv)[:, sl].astype(bf)),
            "wo": np.ascontiguousarray(np.asarray(wo)[sl, :].astype(bf)),
            "bqkT": np.ascontiguousarray(bqk),
            "bv": np.ascontiguousarray(np.asarray(bv)[sl].astype(bf)).reshape(1, DG),
            "masks": mask,
        })
    return in_maps


def combine_outputs(results, bo):
    """Sum the two row-parallel partials per batch and add the output bias."""
    out = np.empty((B, S, D), np.float32)
    for b in range(B):
        out[b] = results[2 * b]["y"] + results[2 * b + 1]["y"] + np.asarray(bo)[None, :]
    return out


_NC_CACHE = {}


def kernel(x, q, k, v, mask, wq, bq, wk, bk, wv, bv, wo, bo):
    # x is unused (overwritten in the reference forward); mask is the causal
    # tril mask, which is hardcoded in the on-device masking.
    if "nc" not in _NC_CACHE:
        _NC_CACHE["nc"] = build_program()
    nc = _NC_CACHE["nc"]
    in_maps = make_inputs(q, k, v, wq, bq, wk, bk, wv, bv, wo)
    r = run_bass_kernel_spmd(nc, in_maps, core_ids=list(range(8)))
    return combine_outputs(r.results, bo)


# revision 5
# speedup vs baseline: 277.7233x; 1.2954x over previous
"""Multi-head attention (B=4, S=2048, D=1024, H=16, causal) on 8 TRN2 NeuronCores.

Sharding: batch (4) x head-group (2 groups of 8 heads) = 8 cores.
Megatron-style: wq/wk/wv column-parallel, wo row-parallel; the 2-way partial-sum
of the row-parallel output projection is folded into the host-side unshard.

v2: all matmul operands bf16 (host-side cast; output tolerance 2e-2 dwarfs the
~4e-3 bf16 rounding).  bf16 enables Fast Weight Load so the per-matmul
LDWEIGHTS hides behind the matmul stream, and halves DMA + SBUF traffic.
Staging DMAs are one 3D-AP transfer per (tensor, chunk) and are emitted in
first-use order so the first projection starts ~5us in instead of ~45us.
q/k bias-adds are folded into the PSUM->SBUF copy on DVE (per-partition scalar
broadcast) instead of burning 512-row PE matmuls.

Per-core algorithm (heads h in the core's group, q-chunks of 512 queries):
  QT[dk, s], KT[dk, s] = (x @ w + b)^T via PE matmuls on host-pre-transposed
  inputs; V[s, dv] likewise, with a ones-column appended per head so that the
  PV matmul also produces softmax denominators.
  scoresT[k, q] = KT-slices x QT (two heads packed in the 128-partition dim,
  concurrent via PE row tiling since dk=64).
  E = exp(scoresT/8) on ACT (no max-subtraction needed: scores ~ N(0,1)).
  Causality: fully-masked key-blocks are never computed; diagonal-crossing
  blocks are trapezoid-sliced to their live q-range and only the leading
  128-column triangle gets a mask multiply.
  ctxT[dv, q] accumulates V-slices x E in PSUM; row 64 = sum(E).
  Normalize: approx-reciprocal (DVE) -> partition_broadcast (GPSIMD) ->
  multiply (DVE).
  y_partial[s, do] = sum over head-pairs of ctxT-slices x wo-rows (PSUM accum).
"""
import sys
import numpy as np

sys.path.insert(0, "/opt/trn_rl_repo")

from contextlib import ExitStack

import ml_dtypes

import concourse.bacc as bacc
import concourse.tile as tile
from concourse import mybir
from concourse.bass_utils import run_bass_kernel_spmd

F32 = mybir.dt.float32
BF16 = mybir.dt.bfloat16
BF16NP = ml_dtypes.bfloat16

B, S, D, H = 4, 2048, 1024, 16
DK = D // H          # 64
HG = H // 2          # 8 heads per core
DG = HG * DK         # 512 columns per core group
SC = 512             # query-chunk width
KB = 128             # key-block height
N_SC = S // SC       # 4
N_KB = S // KB       # 16
N_DM = D // 128      # 8 contraction tiles for projections
N_PAIR = HG // 2     # 4 head pairs per core
EXPSCALE = 1.0 / 8.0  # 1/sqrt(DK)


def build_program():
    """Emit the SPMD Bass program (identical on all 8 cores)."""
    nc = bacc.Bacc("TRN2", target_bir_lowering=False, debug=False)

    qT_in = nc.dram_tensor("qT", [D, S], BF16, kind="ExternalInput").ap()
    kT_in = nc.dram_tensor("kT", [D, S], BF16, kind="ExternalInput").ap()
    vT_in = nc.dram_tensor("vT", [D, S], BF16, kind="ExternalInput").ap()
    wq_in = nc.dram_tensor("wq", [D, DG], BF16, kind="ExternalInput").ap()
    wk_in = nc.dram_tensor("wk", [D, DG], BF16, kind="ExternalInput").ap()
    wv_in = nc.dram_tensor("wv", [D, DG], BF16, kind="ExternalInput").ap()
    wo_in = nc.dram_tensor("wo", [DG, D], BF16, kind="ExternalInput").ap()
    # bqkT[:, 0:4] = bq pair-columns, [:, 4:8] = bk pair-columns
    bqkT_in = nc.dram_tensor("bqkT", [128, 2 * N_PAIR], F32,
                             kind="ExternalInput").ap()
    bv_in = nc.dram_tensor("bv", [1, DG], BF16, kind="ExternalInput").ap()
    # leading-triangle causal mask: mask[ki, qj] = (qj >= ki), [128, 128]
    mask_in = nc.dram_tensor("masks", [KB, KB], BF16, kind="ExternalInput").ap()
    y_out = nc.dram_tensor("y", [S, D], F32, kind="ExternalOutput").ap()

    with tile.TileContext(nc) as tc, ExitStack() as ctx:
        stage = ctx.enter_context(tc.tile_pool(name="stage", bufs=6))
        wpool = ctx.enter_context(tc.tile_pool(name="wpool", bufs=1))
        qtpool = ctx.enter_context(tc.tile_pool(name="qtpool", bufs=8))
        ktpool = ctx.enter_context(tc.tile_pool(name="ktpool", bufs=1))
        vpool = ctx.enter_context(tc.tile_pool(name="vpool", bufs=1))
        epool = ctx.enter_context(tc.tile_pool(name="epool", bufs=3))
        cpool = ctx.enter_context(tc.tile_pool(name="cpool", bufs=4))
        mpool = ctx.enter_context(tc.tile_pool(name="mpool", bufs=1))
        ypool = ctx.enter_context(tc.tile_pool(name="ypool", bufs=2))
        rpool = ctx.enter_context(tc.tile_pool(name="rpool", bufs=2))
        bpool = ctx.enter_context(tc.tile_pool(name="bpool", bufs=2))
        onepool = ctx.enter_context(tc.tile_pool(name="onepool", bufs=1))
        pspool = ctx.enter_context(tc.tile_pool(name="pspool", bufs=1, space="PSUM"))

        # ---- persistent data regions ----
        # KT: per (head-pair, s-chunk) tile [128, SC]; rows 0:64 head 2p.
        KT = [[ktpool.tile([128, SC], BF16, name=f"KT{p}_{sc}")
               for sc in range(N_SC)] for p in range(N_PAIR)]
        # V: per key-block tile [128, HG, 65]; per head 64 value cols + ones col.
        V = [vpool.tile([128, HG, 65], BF16, name=f"V{kb}") for kb in range(N_KB)]

        # weights: one 3D tile per projection, w_sb[:, dm, :] = rows dm*128..+128
        w_sb = {}
        wo_sb = wpool.tile([128, N_PAIR, D], BF16, name="wo_sb")
        ones_sb = onepool.tile([1, 128], BF16, name="ones_sb")
        bqk_sb = onepool.tile([128, 2 * N_PAIR], F32, name="bqk_sb")
        bv_sb = onepool.tile([1, DG], BF16, name="bv_sb")
        mask_sb = mpool.tile([KB, KB], BF16, name="mask_sb")

        def stage_chunk(nm, xT_in, sc):
            # [128, dm, col] <- DRAM rows (p + 128*dm), cols sc*SC..+SC
            t = stage.tile([128, N_DM, SC], BF16, name=f"{nm}s{sc}", tag="stage")
            src = xT_in.rearrange("(dm p) s -> p dm s", p=128)
            nc.sync.dma_start(t[:], src[:, :, sc * SC:(sc + 1) * SC])
            return t

        # ---- DMA / init order: first use first ----
        w_sb["q"] = wpool.tile([128, N_DM, DG], BF16, name="wq_sb")
        nc.sync.dma_start(w_sb["q"][:], wq_in.rearrange("(dm p) c -> p dm c", p=128))
        xq0 = stage_chunk("q", qT_in, 0)
        nc.sync.dma_start(bqk_sb[:], bqkT_in[:])
        w_sb["k"] = wpool.tile([128, N_DM, DG], BF16, name="wk_sb")
        nc.sync.dma_start(w_sb["k"][:], wk_in.rearrange("(dm p) c -> p dm c", p=128))
        xk0 = stage_chunk("k", kT_in, 0)
        w_sb["v"] = wpool.tile([128, N_DM, DG], BF16, name="wv_sb")
        nc.sync.dma_start(w_sb["v"][:], wv_in.rearrange("(dm p) c -> p dm c", p=128))
        nc.sync.dma_start(bv_sb[:], bv_in[:])
        xv0 = stage_chunk("v", vT_in, 0)
        nc.sync.dma_start(mask_sb[:], mask_in[:])
        nc.sync.dma_start(wo_sb[:], wo_in.rearrange("(p r) c -> r p c", r=128))
        nc.gpsimd.memset(ones_sb[:], 1.0)
        for kb in range(N_KB):
            nc.gpsimd.memset(V[kb][:, :, 64:65], 1.0)

        def ps_small(name):
            return pspool.tile([128, SC], F32, name=name, tag="psa", bufs=2)

        def proj_qk(nm, xs, bcol, dst_tiles):
            # dst_tiles[p] <- [128, SC] transposed projection of this chunk,
            # bias added during the PSUM->SBUF copy (bcol = [128,1] slice).
            for p in range(N_PAIR):
                ps = ps_small(f"ps_{nm}")
                for dm in range(N_DM):
                    nc.tensor.matmul(
                        ps[:],
                        w_sb[nm][:, dm, p * 128:(p + 1) * 128],
                        xs[:, dm, :],
                        start=(dm == 0), stop=(dm == N_DM - 1),
                    )
                nc.vector.tensor_tensor(
                    dst_tiles[p][:], ps[:],
                    bcol[:, p:p + 1].broadcast_to([128, SC]),
                    mybir.AluOpType.add,
                )

        def proj_chunk(sc, xq, xk, xv, QTc):
            proj_qk("q", xq, bqk_sb[:, 0:N_PAIR], QTc)
            proj_qk("k", xk, bqk_sb[:, N_PAIR:2 * N_PAIR],
                    [KT[p][sc] for p in range(N_PAIR)])
            for sb in range(4):
                ps = ps_small("ps_v")
                nc.tensor.matmul(
                    ps[:], ones_sb[0:1, :], bv_sb[0:1, :],
                    start=True, stop=False,
                )
                for dm in range(N_DM):
                    nc.tensor.matmul(
                        ps[:],
                        xv[:, dm, sb * 128:(sb + 1) * 128],
                        w_sb["v"][:, dm, :],
                        start=False, stop=(dm == N_DM - 1),
                    )
                kb = sc * 4 + sb
                nc.vector.tensor_copy(
                    V[kb][:, :, 0:64], ps[:].rearrange("p (h d) -> p h d", h=HG)
                )

        def attention(qc, QTc):
            kbmax = 4 * (qc + 1)
            ctx_pairs = []
            for p in range(N_PAIR):
                h0, h1 = 2 * p, 2 * p + 1
                ctx0 = pspool.tile([65, SC], F32, name="ctx0", tag="psctx0", bufs=1)
                ctx1 = pspool.tile([65, SC], F32, name="ctx1", tag="psctx1", bufs=1)
                for kb in range(kbmax):
                    j = kb - 4 * qc  # >=0: diagonal-crossing block
                    off = max(j, 0) * KB  # live q-range is [off, SC)
                    kt = KT[p][kb // 4]
                    kcol = (kb % 4) * KB
                    scps = pspool.tile([128, 2 * SC], F32, name="scps", tag="pssc",
                                       bufs=2)
                    # scoresT: two heads concurrently via PE row groups
                    nc.tensor.matmul(
                        scps[:, off:SC],
                        kt[0:64, kcol:kcol + KB],
                        QTc[p][0:64, off:SC],
                        start=True, stop=True,
                    )
                    nc.tensor.matmul(
                        scps[:, SC + off:2 * SC],
                        kt[64:128, kcol:kcol + KB],
                        QTc[p][64:128, off:SC],
                        start=True, stop=True,
                    )
                    e = epool.tile([128, 2, SC], BF16, name="e", tag="e")
                    if off == 0:
                        nc.scalar.activation(
                            e[:], scps[:].rearrange("p (h q) -> p h q", h=2),
                            mybir.ActivationFunctionType.Exp,
                            scale=EXPSCALE,
                        )
                    else:
                        nc.scalar.activation(
                            e[:, :, off:SC],
                            scps[:].rearrange("p (h q) -> p h q", h=2)[:, :, off:SC],
                            mybir.ActivationFunctionType.Exp, scale=EXPSCALE,
                        )
                    if j >= 0:  # mask the leading 128-col triangle
                        nc.vector.tensor_tensor(
                            e[:, :, off:off + KB], e[:, :, off:off + KB],
                            mask_sb[:].unsqueeze(1).broadcast_to([KB, 2, KB]),
                            mybir.AluOpType.mult,
                        )
                    first, last = kb == 0, kb == kbmax - 1
                    nc.tensor.matmul(
                        ctx0[:, off:SC], V[kb][:, h0, :], e[:, 0, off:SC],
                        start=first, stop=last,
                    )
                    nc.tensor.matmul(
                        ctx1[:, off:SC], V[kb][:, h1, :], e[:, 1, off:SC],
                        start=first, stop=last,
                    )
                # normalize: ctx rows 0:64 / ctx row 64; the two heads' chains
                # are interleaved so DVE/GPSIMD pipeline them.
                cp = cpool.tile([128, SC], BF16, name="cp", tag="ctx")
                # reciprocal_approx_fast mis-reads PSUM on HW: hop via SBUF
                srows, recs, rbs = [], [], []
                for i, cps in ((0, ctx0), (1, ctx1)):
                    srow = rpool.tile([1, SC], F32, name=f"srow{i}", tag="rec",
                                      bufs=4)
                    nc.vector.tensor_copy(srow[:], cps[64:65, :])
                    srows.append(srow)
                for i in range(2):
                    rec = rpool.tile([1, SC], F32, name=f"rec{i}", tag="rec",
                                     bufs=4)
                    nc.vector.reciprocal_approx_fast(rec[:], srows[i][:])
                    recs.append(rec)
                for i in range(2):
                    rb = bpool.tile([64, SC], F32, name=f"rb{i}", tag="rb", bufs=2)
                    nc.gpsimd.partition_broadcast(rb[:], recs[i][:])
                    rbs.append(rb)
                for i, cps in ((0, ctx0), (1, ctx1)):
                    nc.vector.tensor_tensor(
                        cp[i * 64:(i + 1) * 64, :], cps[0:64, :], rbs[i][:],
                        mybir.AluOpType.mult,
                    )
                ctx_pairs.append(cp)
            return ctx_pairs

        def yproj(qc, ctx_pairs):
            for sb in range(4):
                yst = ypool.tile([128, D], F32, name="yst", tag="y")
                for dc in range(2):
                    yps = ps_small("yps")
                    for p in range(N_PAIR):
                        nc.tensor.matmul(
                            yps[:],
                            ctx_pairs[p][:, sb * 128:(sb + 1) * 128],
                            wo_sb[:, p, dc * SC:(dc + 1) * SC],
                            start=(p == 0), stop=(p == N_PAIR - 1),
                        )
                    nc.vector.tensor_copy(yst[:, dc * SC:(dc + 1) * SC], yps[:])
                row = qc * SC + sb * 128
                nc.sync.dma_start(y_out[row:row + 128, :], yst[:])

        # ---- main loop, software-pipelined: proj(sc+1) is emitted between
        # attention(sc) and yproj(sc) so the scheduler can fill the PE while
        # the ACT-bound attention tail + normalize chains drain.
        QTs = {0: [qtpool.tile([128, SC], BF16, name=f"QT{p}_0", tag="qtc")
                   for p in range(N_PAIR)]}
        proj_chunk(0, xq0, xk0, xv0, QTs[0])
        for sc in range(N_SC):
            if sc + 1 < N_SC:
                nxq = stage_chunk("q", qT_in, sc + 1)
                nxk = stage_chunk("k", kT_in, sc + 1)
                nxv = stage_chunk("v", vT_in, sc + 1)
            ctx_pairs = attention(sc, QTs[sc])
            if sc + 1 < N_SC:
                QTs[sc + 1] = [qtpool.tile([128, SC], BF16,
                                           name=f"QT{p}_{sc+1}", tag="qtc")
                               for p in range(N_PAIR)]
                proj_chunk(sc + 1, nxq, nxk, nxv, QTs[sc + 1])
                del QTs[sc]
            yproj(sc, ctx_pairs)

    nc.compile()
    return nc


def make_inputs(q, k, v, wq, bq, wk, bk, wv, bv, wo):
    """Host-side shard + layout prep. Returns list of 8 per-core input dicts."""
    bf = BF16NP
    qj = np.arange(KB)[None, :]
    ki = np.arange(KB)[:, None]
    mask = np.ascontiguousarray((qj >= ki).astype(bf))

    qT = [np.ascontiguousarray(np.asarray(q[b]).T.astype(bf)) for b in range(B)]
    kT = [np.ascontiguousarray(np.asarray(k[b]).T.astype(bf)) for b in range(B)]
    vT = [np.ascontiguousarray(np.asarray(v[b]).T.astype(bf)) for b in range(B)]

    in_maps = []
    for c in range(8):
        b, g = c // 2, c % 2
        sl = slice(g * DG, (g + 1) * DG)
        bqk = np.stack([np.asarray(bq[sl]).astype(np.float32)
                        .reshape(N_PAIR, 128)[p] for p in range(N_PAIR)]
                       + [np.asarray(bk[sl]).astype(np.float32)
                          .reshape(N_PAIR, 128)[p] for p in range(N_PAIR)],
                       axis=1)
        in_maps.append({
            "qT": qT[b], "kT": kT[b], "vT": vT[b],
            "wq": np.ascontiguousarray(np.asarray(wq)[:, sl].astype(bf)),
            "wk": np.ascontiguousarray(np.asarray(wk)[:, sl].astype(bf)),
            "wv": np.ascontiguousarray(np.asarray(wv)[:, sl].astype(bf)),
            "wo": np.ascontiguousarray(np.asarray(wo)[sl, :].astype(bf)),
            "bqkT": np.ascontiguousarray(bqk),
            "bv": np.ascontiguousarray(np.asarray(bv)[sl].astype(bf)).reshape(1, DG),
            "masks": mask,
        })
    return in_maps


def combine_outputs(results, bo):
    """Sum the two row-parallel partials per batch and add the output bias."""
    out = np.empty((B, S, D), np.float32)
    for b in range(B):
        out[b] = results[2 * b]["y"] + results[2 * b + 1]["y"] + np.asarray(bo)[None, :]
    return out


_NC_CACHE = {}


def kernel(x, q, k, v, mask, wq, bq, wk, bk, wv, bv, wo, bo):
    # x is unused (overwritten in the reference forward); mask is the causal
    # tril mask, which is hardcoded in the on-device masking.
    if "nc" not in _NC_CACHE:
        _NC_CACHE["nc"] = build_program()
    nc = _NC_CACHE["nc"]
    in_maps = make_inputs(q, k, v, wq, bq, wk, bk, wv, bv, wo)
    r = run_bass_kernel_spmd(nc, in_maps, core_ids=list(range(8)))
    return combine_outputs(r.results, bo)
